# revision 8
# baseline (speedup 1.0000x reference)
"""Trainium2 Bass kernel for windowed multi-head attention with dynamic
position bias (sparse_attention, B=2, H=W=256, 8x32 windows, 6 heads, d=32).

Strategy (data-parallel over windows, 8 cores x 64 windows):
  host:   im2win + shard + pre-transpose Q,K to [c, token] layout, fold the
          attention scale into Q, evaluate the tiny pos-bias MLP + gather to
          the 6 per-head (256,256) bias matrices.
  device: per (window, head):  S^T + B^T accumulated in PSUM
            - bias injected via identity matmul (bf16 stream, exact)
            - Q K^T as 4x row-tiled fp32r matmuls (contract dim 32)
          softmax without max-subtraction (logits are ~N(0,1)):
            - single ScalarE exp pass PSUM->SBUF bf16 over 3 banks at a time
          P V with P^T as the (FWL bf16) stationary and ones-augmented V as
          the moving operand -> output lands [q, d] with row sums in col 32;
          VectorE reciprocal + broadcast-multiply normalizes straight into a
          dense [128, 192] tile that DMAs out with 768B/partition runs.
"""

import sys

sys.path.insert(0, "/opt/trn_rl_repo")

import numpy as np

import concourse.bass as bass
import concourse.tile as tile
from concourse import mybir
from concourse.alu_op_type import AluOpType
from concourse.bass_utils import run_bass_kernel_spmd

F32 = mybir.dt.float32
F32R = mybir.dt.float32r
BF16 = mybir.dt.bfloat16
EXP = mybir.ActivationFunctionType.Exp

N_CORES = 8
B, H, W = 2, 256, 256
H_SP, W_SP = 8, 32
NUM_HEADS = 6
DIM = 192
HEAD_DIM = 32
SCALE = HEAD_DIM ** -0.5
N = H_SP * W_SP                     # 256 tokens / window
NW_TOTAL = B * (H // H_SP) * (W // W_SP)   # 512 windows
NW = NW_TOTAL // N_CORES            # 64 windows / core


# --------------------------------------------------------------------------
# device program
# --------------------------------------------------------------------------
def build_program(nw=NW):
    from concourse import bacc
    nc = bacc.Bacc("TRN2", target_bir_lowering=False, debug=False)

    qT = nc.dram_tensor("qT", [nw, DIM, N], F32R, kind="ExternalInput").ap()
    kT = nc.dram_tensor("kT", [nw, DIM, N], F32R, kind="ExternalInput").ap()
    vA = nc.dram_tensor("vA", [nw, 2, 128, 198], BF16, kind="ExternalInput").ap()
    biasT = nc.dram_tensor("biasT", [NUM_HEADS, 128, 512], BF16,
                           kind="ExternalInput").ap()
    ident = nc.dram_tensor("ident", [128, 128], BF16, kind="ExternalInput").ap()
    outw = nc.dram_tensor("outw", [nw, 2, 128, DIM], F32,
                          kind="ExternalOutput").ap()

    with tile.TileContext(nc) as tc:
        _emit(nc, tc, nw, qT, kT, vA, biasT, ident, outw)
    nc.compile()
    return nc


def _emit(nc, tc, nw, qT, kT, vA, biasT, ident, outw):
    from contextlib import ExitStack
    ctx = ExitStack()

    # resident tensors: per-head bias panels [128, 512] (free = kc*256 + q)
    # and the identity used to stream them into PSUM.
    bias_sb = nc.alloc_sbuf_tensor("bias_sb", [128, NUM_HEADS * 512], BF16).ap()
    id_sb = nc.alloc_sbuf_tensor("id_sb", [128, 128], BF16).ap()
    nc.sync.dma_start(
        bias_sb.rearrange("p (h f) -> p h f", h=NUM_HEADS),
        biasT.rearrange("h p f -> p h f"),
    )
    nc.sync.dma_start(id_sb, ident)

    pin = ctx.enter_context(tc.tile_pool(name="pin", bufs=3))
    pps = ctx.enter_context(tc.tile_pool(name="pps", bufs=2, space="PSUM"))
    ppt = ctx.enter_context(tc.tile_pool(name="ppt", bufs=2))
    pout = ctx.enter_context(tc.tile_pool(name="pout", bufs=4))

    for w in range(nw):
        qa = pin.tile([128, N], F32R, tag="qa")
        nc.sync.dma_start(qa, qT[w, 0:128])
        qb = pin.tile([64, N], F32R, tag="qb")
        nc.sync.dma_start(qb, qT[w, 128:192])
        ka = pin.tile([128, N], F32R, tag="ka")
        nc.sync.dma_start(ka, kT[w, 0:128])
        kb = pin.tile([64, N], F32R, tag="kb")
        nc.sync.dma_start(kb, kT[w, 128:192])
        va = pin.tile([128, 396], BF16, tag="va")
        nc.sync.dma_start(va[:, 0:198], vA[w, 0])
        nc.sync.dma_start(va[:, 198:396], vA[w, 1])

        # S^T(+bias) accumulators: two 3-bank groups, head h at cols 512*(h%3)
        sA = pps.tile([128, 1536], F32, tag="s")
        sB = pps.tile([128, 1536], F32, tag="s")

        def bank(h):
            s = sA if h < 3 else sB
            return s[:, 512 * (h % 3): 512 * (h % 3) + 512]

        # 1) exact bias -> PSUM via identity matmul (one N=512 MM per head)
        for h in range(NUM_HEADS):
            nc.tensor.matmul(
                bank(h), lhsT=id_sb, rhs=bias_sb[:, 512 * h: 512 * h + 512],
                start=True, stop=False, skip_group_check=True,
            )
        # 2) S^T += K^T.T @ Q^T, fp32r, 4x row-tiled over head quadrants
        for kc in (0, 1):
            for h in range(NUM_HEADS):
                hp = h if h < 4 else h - 4
                ktile = ka if h < 4 else kb
                qtile = qa if h < 4 else qb
                nc.tensor.matmul(
                    bank(h)[:, 256 * kc: 256 * kc + 256],
                    lhsT=ktile[32 * hp: 32 * hp + 32,
                               128 * kc: 128 * kc + 128],
                    rhs=qtile[32 * hp: 32 * hp + 32, :],
                    start=False, stop=(kc == 1),
                    tile_position=(32 * hp, 0), skip_group_check=True,
                )

        # 3) softmax numerator: exp over 3 banks at a time, PSUM -> SBUF bf16
        ptA = ppt.tile([128, 1536], BF16, tag="pt")
        ptB = ppt.tile([128, 1536], BF16, tag="pt")
        nc.scalar.activation(ptA, sA, EXP)
        nc.scalar.activation(ptB, sB, EXP)

        # 4) P V with ones-augmented V; P^T chunks are the stationaries
        for qc in (0, 1):
            pv = pps.tile([128, 198], F32, tag="pv", bufs=2)
            for h in range(NUM_HEADS):
                pt = ptA if h < 3 else ptB
                base = 512 * (h % 3)
                for kc in (0, 1):
                    nc.tensor.matmul(
                        pv[:, 33 * h: 33 * h + 33],
                        lhsT=pt[:, base + 256 * kc + 128 * qc:
                                base + 256 * kc + 128 * qc + 128],
                        rhs=va[:, 198 * kc + 33 * h: 198 * kc + 33 * h + 33],
                        start=(kc == 0), stop=(kc == 1),
                        skip_group_check=True,
                    )
            # 5) normalize: out[q, (h,d)] = pv[q, (h,d)] * (1 / pv[q, (h,32)])
            pv3 = pv.rearrange("p (h c) -> p h c", c=33)
            rv = pout.tile([128, 8], F32, tag="rv", bufs=4)
            nc.vector.reciprocal(rv[:, 0:NUM_HEADS], pv3[:, :, 32])
            ot = pout.tile([128, DIM], F32, tag="ot", bufs=4)
            nc.vector.tensor_tensor(
                ot.rearrange("p (h c) -> p h c", c=32),
                pv3[:, :, 0:32],
                rv[:, 0:NUM_HEADS].unsqueeze(-1).broadcast_to(
                    [128, NUM_HEADS, 32]),
                op=AluOpType.mult,
            )
            nc.sync.dma_start(outw[w, qc], ot)
    ctx.close()


# --------------------------------------------------------------------------
# host side
# --------------------------------------------------------------------------
def _layer_norm(x, g, b, eps=1e-5):
    m = x.mean(-1, keepdims=True)
    v = x.var(-1, keepdims=True)
    return (x - m) / np.sqrt(v + eps) * g + b


def compute_bias(rpe_biases, rel_index, pos_proj_w, pos_proj_b, ln1_g, ln1_b,
                 fc1_w, fc1_b, ln2_g, ln2_b, fc2_w, fc2_b, ln3_g, ln3_b,
                 fc3_w, fc3_b):
    """pos-bias MLP + gather, in fp64 on host -> (6, 256, 256) fp32 [h, q, k]."""
    f8 = np.float64
    p = rpe_biases.astype(f8) @ pos_proj_w.astype(f8) + pos_proj_b.astype(f8)
    p = np.maximum(_layer_norm(p, ln1_g.astype(f8), ln1_b.astype(f8)), 0)
    p = p @ fc1_w.astype(f8) + fc1_b.astype(f8)
    p = np.maximum(_layer_norm(p, ln2_g.astype(f8), ln2_b.astype(f8)), 0)
    p = p @ fc2_w.astype(f8) + fc2_b.astype(f8)
    p = np.maximum(_layer_norm(p, ln3_g.astype(f8), ln3_b.astype(f8)), 0)
    pos = p @ fc3_w.astype(f8) + fc3_b.astype(f8)          # (num_biases, 6)
    rel = pos[np.asarray(rel_index).reshape(-1)]
    return np.ascontiguousarray(
        rel.reshape(N, N, NUM_HEADS).transpose(2, 0, 1)).astype(np.float32)


def im2win(x):
    """(B, L, C) -> (512, 256, C) windows in (b, hb, wb) / (hs, ws) order."""
    x = x.reshape(B, H // H_SP, H_SP, W // W_SP, W_SP, DIM)
    x = x.transpose(0, 1, 3, 2, 4, 5)
    return np.ascontiguousarray(x.reshape(NW_TOTAL, N, DIM))


def prep_inputs(qkv, bias):
    """Build the full (unsharded) device arrays; shard by slicing windows."""
    import ml_dtypes
    bf = ml_dtypes.bfloat16

    q = im2win(np.asarray(qkv[0]))
    k = im2win(np.asarray(qkv[1]))
    v = im2win(np.asarray(qkv[2]))

    qTf = np.ascontiguousarray((q * np.float32(SCALE)).transpose(0, 2, 1))
    kTf = np.ascontiguousarray(k.transpose(0, 2, 1))      # (512, 192, 256)

    vr = v.reshape(NW_TOTAL, 2, 128, NUM_HEADS, HEAD_DIM)
    ones = np.ones((NW_TOTAL, 2, 128, NUM_HEADS, 1), np.float32)
    vAf = np.concatenate([vr, ones], -1).reshape(NW_TOTAL, 2, 128, 198)
    vAf = vAf.astype(bf)

    # biasT[h][k_local, 256*kc + q] = bias[h, q, 128*kc + k_local]
    bt = bias.transpose(0, 2, 1).reshape(NUM_HEADS, 2, 128, N)   # h, kc, k, q
    btT = np.ascontiguousarray(bt.transpose(0, 2, 1, 3)).reshape(
        NUM_HEADS, 128, 512).astype(bf)

    identity = np.eye(128, dtype=np.float32).astype(bf)
    return qTf, kTf, vAf, btT, identity


def _run(qkv, rpe_biases, rel_index, params, trace=False, **spmd_kwargs):
    qkv = np.asarray(qkv, np.float32)
    bias = compute_bias(np.asarray(rpe_biases), np.asarray(rel_index), **params)
    qTf, kTf, vAf, btT, identity = prep_inputs(qkv, bias)

    nc = build_program(NW)
    in_maps = []
    for c in range(N_CORES):
        s = slice(c * NW, (c + 1) * NW)
        in_maps.append({
            "qT": qTf[s], "kT": kTf[s], "vA": vAf[s],
            "biasT": btT, "ident": identity,
        })
    res = run_bass_kernel_spmd(nc, in_maps, core_ids=list(range(N_CORES)),
                               trace=trace, **spmd_kwargs)

    outw = np.stack([res.results[c]["outw"] for c in range(N_CORES)])
    return unwindow(outw.reshape(NW_TOTAL, N, DIM)), res


def kernel(qkv, H=None, W=None, rpe_biases=None, rel_index=None, **params):
    return _run(qkv, rpe_biases, rel_index, params)[0]


def unwindow(x):
    """(512, 256, 192) -> (B, H, W, C)"""
    x = x.reshape(B, H // H_SP, W // W_SP, H_SP, W_SP, DIM)
    x = x.transpose(0, 1, 3, 2, 4, 5)
    return np.ascontiguousarray(x.reshape(B, H, W, DIM))


# revision 13
# speedup vs baseline: 1.8166x; 1.8166x over previous
"""Trainium2 Bass kernel for windowed multi-head attention with dynamic
position bias (sparse_attention, B=2, H=W=256, 8x32 windows, 6 heads, d=32).

Strategy (data-parallel over windows, 8 cores x 64 windows):
  host:   im2win + shard + pre-transpose Q,K to [c, token] layout, fold the
          attention scale into Q, evaluate the tiny pos-bias MLP + gather to
          the 6 per-head (256,256) bias matrices.
  device: per (window, head):  S^T + B^T accumulated in PSUM
            - bias injected via identity matmul (bf16 stream, exact)
            - Q K^T as 4x row-tiled fp32r matmuls (contract dim 32)
          softmax without max-subtraction (logits are ~N(0,1)):
            - single ScalarE exp pass PSUM->SBUF bf16 over 3 banks at a time
          P V with P^T as the (FWL bf16) stationary and ones-augmented V as
          the moving operand -> output lands [q, d] with row sums in col 32;
          VectorE reciprocal + broadcast-multiply normalizes straight into a
          dense [128, 192] tile that DMAs out with 768B/partition runs.
"""

import sys

sys.path.insert(0, "/opt/trn_rl_repo")

import numpy as np

import concourse.bass as bass
import concourse.tile as tile
from concourse import mybir
from concourse.alu_op_type import AluOpType
from concourse.bass_utils import run_bass_kernel_spmd

F32 = mybir.dt.float32
F32R = mybir.dt.float32r
BF16 = mybir.dt.bfloat16
EXP = mybir.ActivationFunctionType.Exp

N_CORES = 8
B, H, W = 2, 256, 256
H_SP, W_SP = 8, 32
NUM_HEADS = 6
DIM = 192
HEAD_DIM = 32
SCALE = HEAD_DIM ** -0.5
N = H_SP * W_SP                     # 256 tokens / window
NW_TOTAL = B * (H // H_SP) * (W // W_SP)   # 512 windows
NW = NW_TOTAL // N_CORES            # 64 windows / core


# --------------------------------------------------------------------------
# device program
# --------------------------------------------------------------------------
WG = 8     # windows per input slab (8KB/partition DMA runs)
OG = 4     # windows per output slab (6KB/partition DMA runs)


def build_program(nw=NW):
    from concourse import bacc
    nc = bacc.Bacc("TRN2", target_bir_lowering=False, debug=False)

    # window-major layouts: per-partition contiguous runs span WG windows
    qT = nc.dram_tensor("qT", [DIM, nw * N], F32R, kind="ExternalInput").ap()
    kT = nc.dram_tensor("kT", [DIM, nw * N], F32R, kind="ExternalInput").ap()
    vA = nc.dram_tensor("vA", [128, nw * 396], BF16, kind="ExternalInput").ap()
    biasT = nc.dram_tensor("biasT", [NUM_HEADS, 128, 512], BF16,
                           kind="ExternalInput").ap()
    ident = nc.dram_tensor("ident", [128, 128], BF16, kind="ExternalInput").ap()
    outw = nc.dram_tensor("outw", [128, nw * 2 * DIM], F32,
                          kind="ExternalOutput").ap()

    with tile.TileContext(nc) as tc:
        _emit(nc, tc, nw, qT, kT, vA, biasT, ident, outw)
    nc.compile()
    return nc


def _emit(nc, tc, nw, qT, kT, vA, biasT, ident, outw):
    from contextlib import ExitStack
    ctx = ExitStack()

    # resident tensors: per-head bias panels [128, 512] (free = kc*256 + q)
    # and the identity used to stream them into PSUM.
    bias_sb = nc.alloc_sbuf_tensor("bias_sb", [128, NUM_HEADS * 512], BF16).ap()
    id_sb = nc.alloc_sbuf_tensor("id_sb", [128, 128], BF16).ap()
    nc.sync.dma_start(
        bias_sb.rearrange("p (h f) -> p h f", h=NUM_HEADS),
        biasT.rearrange("h p f -> p h f"),
    )
    nc.sync.dma_start(id_sb, ident)

    pin = ctx.enter_context(tc.tile_pool(name="pin", bufs=2))
    pps = ctx.enter_context(tc.tile_pool(name="pps", bufs=2, space="PSUM"))
    ppt = ctx.enter_context(tc.tile_pool(name="ppt", bufs=2))
    pout = ctx.enter_context(tc.tile_pool(name="pout", bufs=4))

    qa = qb = ka = kb = va = ob = None
    for w in range(nw):
        if w % WG == 0:   # load an input slab of WG windows in 5 big DMAs
            g = w * N
            qa = pin.tile([128, WG * N], F32R, tag="qa")
            nc.sync.dma_start(qa, qT[0:128, g:g + WG * N])
            qb = pin.tile([64, WG * N], F32R, tag="qb")
            nc.sync.dma_start(qb, qT[128:192, g:g + WG * N])
            ka = pin.tile([128, WG * N], F32R, tag="ka")
            nc.sync.dma_start(ka, kT[0:128, g:g + WG * N])
            kb = pin.tile([64, WG * N], F32R, tag="kb")
            nc.sync.dma_start(kb, kT[128:192, g:g + WG * N])
            va = pin.tile([128, WG * 396], BF16, tag="va")
            nc.sync.dma_start(va, vA[:, w * 396:(w + WG) * 396])
        if w % OG == 0:   # fresh output slab
            ob = pout.tile([128, OG * 2 * DIM], F32, tag="ob", bufs=3)
        wq = (w % WG) * N          # this window's offset in the input slabs
        wv = (w % WG) * 396

        # S^T(+bias) accumulators: two 3-bank groups, head h at cols 512*(h%3)
        sA = pps.tile([128, 1536], F32, tag="s")
        sB = pps.tile([128, 1536], F32, tag="s")

        def bank(h):
            s = sA if h < 3 else sB
            return s[:, 512 * (h % 3): 512 * (h % 3) + 512]

        # 1) exact bias -> PSUM via identity matmul (one N=512 MM per head)
        for h in range(NUM_HEADS):
            nc.tensor.matmul(
                bank(h), lhsT=id_sb, rhs=bias_sb[:, 512 * h: 512 * h + 512],
                start=True, stop=False, skip_group_check=True,
            )
        # 2) S^T += K^T.T @ Q^T, fp32r, 4x row-tiled over head quadrants
        for kc in (0, 1):
            for h in range(NUM_HEADS):
                hp = h if h < 4 else h - 4
                ktile = ka if h < 4 else kb
                qtile = qa if h < 4 else qb
                nc.tensor.matmul(
                    bank(h)[:, 256 * kc: 256 * kc + 256],
                    lhsT=ktile[32 * hp: 32 * hp + 32,
                               wq + 128 * kc: wq + 128 * kc + 128],
                    rhs=qtile[32 * hp: 32 * hp + 32, wq: wq + N],
                    start=False, stop=(kc == 1),
                    tile_position=(32 * hp, 0), skip_group_check=True,
                )

        # 3) softmax numerator: exp over 3 banks at a time, PSUM -> SBUF bf16
        ptA = ppt.tile([128, 1536], BF16, tag="pt")
        ptB = ppt.tile([128, 1536], BF16, tag="pt")
        nc.scalar.activation(ptA, sA, EXP)
        nc.scalar.activation(ptB, sB, EXP)

        # 4) P V with ones-augmented V; P^T chunks are the stationaries
        for qc in (0, 1):
            pv = pps.tile([128, 198], F32, tag="pv", bufs=2)
            for h in range(NUM_HEADS):
                pt = ptA if h < 3 else ptB
                base = 512 * (h % 3)
                for kc in (0, 1):
                    nc.tensor.matmul(
                        pv[:, 33 * h: 33 * h + 33],
                        lhsT=pt[:, base + 256 * kc + 128 * qc:
                                base + 256 * kc + 128 * qc + 128],
                        rhs=va[:, wv + 198 * kc + 33 * h:
                               wv + 198 * kc + 33 * h + 33],
                        start=(kc == 0), stop=(kc == 1),
                        skip_group_check=True,
                    )
            # 5) normalize: out[q, (h,d)] = pv[q, (h,d)] * (1 / pv[q, (h,32)])
            pv3 = pv.rearrange("p (h c) -> p h c", c=33)
            rv = pout.tile([128, 8], F32, tag="rv", bufs=4)
            nc.vector.reciprocal(rv[:, 0:NUM_HEADS], pv3[:, :, 32])
            oslot = ob[:, ((w % OG) * 2 + qc) * DIM:
                       ((w % OG) * 2 + qc) * DIM + DIM]
            nc.vector.tensor_tensor(
                oslot.rearrange("p (h c) -> p h c", c=32),
                pv3[:, :, 0:32],
                rv[:, 0:NUM_HEADS].unsqueeze(-1).broadcast_to(
                    [128, NUM_HEADS, 32]),
                op=AluOpType.mult,
            )
        if w % OG == OG - 1:  # flush the output slab in one DMA
            base = (w - (OG - 1)) * 2 * DIM
            nc.sync.dma_start(outw[:, base: base + OG * 2 * DIM], ob)
    ctx.close()


# --------------------------------------------------------------------------
# host side
# --------------------------------------------------------------------------
def _layer_norm(x, g, b, eps=1e-5):
    m = x.mean(-1, keepdims=True)
    v = x.var(-1, keepdims=True)
    return (x - m) / np.sqrt(v + eps) * g + b


def compute_bias(rpe_biases, rel_index, pos_proj_w, pos_proj_b, ln1_g, ln1_b,
                 fc1_w, fc1_b, ln2_g, ln2_b, fc2_w, fc2_b, ln3_g, ln3_b,
                 fc3_w, fc3_b):
    """pos-bias MLP + gather, in fp64 on host -> (6, 256, 256) fp32 [h, q, k]."""
    f8 = np.float64
    p = rpe_biases.astype(f8) @ pos_proj_w.astype(f8) + pos_proj_b.astype(f8)
    p = np.maximum(_layer_norm(p, ln1_g.astype(f8), ln1_b.astype(f8)), 0)
    p = p @ fc1_w.astype(f8) + fc1_b.astype(f8)
    p = np.maximum(_layer_norm(p, ln2_g.astype(f8), ln2_b.astype(f8)), 0)
    p = p @ fc2_w.astype(f8) + fc2_b.astype(f8)
    p = np.maximum(_layer_norm(p, ln3_g.astype(f8), ln3_b.astype(f8)), 0)
    pos = p @ fc3_w.astype(f8) + fc3_b.astype(f8)          # (num_biases, 6)
    rel = pos[np.asarray(rel_index).reshape(-1)]
    return np.ascontiguousarray(
        rel.reshape(N, N, NUM_HEADS).transpose(2, 0, 1)).astype(np.float32)


def im2win(x):
    """(B, L, C) -> (512, 256, C) windows in (b, hb, wb) / (hs, ws) order."""
    x = x.reshape(B, H // H_SP, H_SP, W // W_SP, W_SP, DIM)
    x = x.transpose(0, 1, 3, 2, 4, 5)
    return np.ascontiguousarray(x.reshape(NW_TOTAL, N, DIM))


def prep_inputs(qkv, bias):
    """Build the full (unsharded) device arrays in window-major layouts so
    per-partition DMA runs span many windows. Shard by slicing axis 1."""
    import ml_dtypes
    bf = ml_dtypes.bfloat16

    q = im2win(np.asarray(qkv[0]))
    k = im2win(np.asarray(qkv[1]))
    v = im2win(np.asarray(qkv[2]))

    # (512, 256, 192) -> [192, 512, 256]: partition-major, windows inner
    qTf = np.ascontiguousarray(
        (q * np.float32(SCALE)).transpose(2, 0, 1))
    kTf = np.ascontiguousarray(k.transpose(2, 0, 1))

    vr = v.reshape(NW_TOTAL, 2, 128, NUM_HEADS, HEAD_DIM)
    ones = np.ones((NW_TOTAL, 2, 128, NUM_HEADS, 1), np.float32)
    vAf = np.concatenate([vr, ones], -1)          # (512, 2, 128, 6, 33)
    # -> [128, 512, 2*198]
    vAf = np.ascontiguousarray(
        vAf.reshape(NW_TOTAL, 2, 128, 198).transpose(2, 0, 1, 3)
    ).reshape(128, NW_TOTAL, 396).astype(bf)

    # biasT[h][k_local, 256*kc + q] = bias[h, q, 128*kc + k_local]
    bt = bias.transpose(0, 2, 1).reshape(NUM_HEADS, 2, 128, N)   # h, kc, k, q
    btT = np.ascontiguousarray(bt.transpose(0, 2, 1, 3)).reshape(
        NUM_HEADS, 128, 512).astype(bf)

    identity = np.eye(128, dtype=np.float32).astype(bf)
    return qTf, kTf, vAf, btT, identity


def _run(qkv, rpe_biases, rel_index, params, trace=False, **spmd_kwargs):
    qkv = np.asarray(qkv, np.float32)
    bias = compute_bias(np.asarray(rpe_biases), np.asarray(rel_index), **params)
    qTf, kTf, vAf, btT, identity = prep_inputs(qkv, bias)

    nc = build_program(NW)
    in_maps = []
    for c in range(N_CORES):
        s = slice(c * NW, (c + 1) * NW)
        in_maps.append({
            "qT": np.ascontiguousarray(qTf[:, s]).reshape(DIM, NW * N),
            "kT": np.ascontiguousarray(kTf[:, s]).reshape(DIM, NW * N),
            "vA": np.ascontiguousarray(vAf[:, s]).reshape(128, NW * 396),
            "biasT": btT, "ident": identity,
        })
    res = run_bass_kernel_spmd(nc, in_maps, core_ids=list(range(N_CORES)),
                               trace=trace, **spmd_kwargs)

    outw = np.stack([res.results[c]["outw"] for c in range(N_CORES)])
    # outw: (8, 128, NW*2*DIM) -> windows (512, 256, DIM)
    x = outw.reshape(N_CORES, 128, NW, 2, DIM).transpose(0, 2, 3, 1, 4)
    return unwindow(x.reshape(NW_TOTAL, N, DIM)), res


def kernel(qkv, H=None, W=None, rpe_biases=None, rel_index=None, **params):
    return _run(qkv, rpe_biases, rel_index, params)[0]


def unwindow(x):
    """(512, 256, 192) -> (B, H, W, C)"""
    x = x.reshape(B, H // H_SP, W // W_SP, H_SP, W_SP, DIM)
    x = x.transpose(0, 1, 3, 2, 4, 5)
    return np.ascontiguousarray(x.reshape(B, H, W, DIM))
